# revision 5
# baseline (speedup 1.0000x reference)
"""CapsuleLayer dynamic-routing kernel for 8x Trainium2 NeuronCores.

Problem: x (256, 8, 1152) f32, W (1, 1152, 10, 16, 8) f32 ->
v (256, 10, 16, 1) f32 via 4 routing iterations.

u_hat (189 MB) is never materialized; each routing iteration is computed
in factorized form directly from x and W:
    s[b, jd]  = sum_{ck} xT[ck, b] * (c_ij[c, j] * W[c, j, d, k])   (PE)
    M[c, jdk] = sum_b x[b, kc] * v[b, jd]                            (PE)
    agr[c, j] = sum_{dk} W[c, j, d, k] * M[c, j, d, k]               (DVE)

Sharding: the routing state is c-sharded. Core r owns c-block r (128 of
the 1152 input capsules); the 9th block is replicated (every core
computes its full M/agreement so its b_ij stays consistent without a
collective, while its s-contribution is k-striped so the sum is counted
once). b_ij, softmax, Wc, M and the agreement never leave the core; the
only cross-core exchange is ONE AllReduce per iteration of the s partial
sums (256 x 160 fp32, 160 KB), which also serves as the full-batch v
broadcast for the next agreement. Matmul operands are rounded to
float32r (TF32) for 4x PE throughput with fp32 PSUM accumulation; moving
dims are padded to 256 so float32r runs at full rate. End-to-end absmax
relative error vs the fp32 oracle is ~3e-4.
"""
import os
import numpy as np

C, J, D, K = 1152, 10, 16, 8
B, NCORES = 256, 8
BS = B // NCORES
CB = C // 128               # 9 c-blocks; blocks 0..7 sharded, block 8 shared
JD = J * D                  # 160
NIT = 4

_CACHE = {}
LAST_RESULTS = None


def _build(ncores=NCORES, nocc=False):
    """Build + compile the per-core program.

    nocc=True: timing variant for TimelineSim -- collectives are replaced
    by equivalent-size local DMAs that keep the serializing dependency.
    """
    from concourse import bass, bacc, tile, mybir

    f32 = mybir.dt.float32
    f32r = mybir.dt.float32r
    nc = bacc.Bacc("TRN2", target_bir_lowering=False, debug=False,
                   num_devices=ncores)

    # per-core inputs (host-sharded):
    #  xTs:  lhsT for my s-block, [c 128, (k 8, h 2, b 128)]
    #  xT8k: lhsT for block-8 k=r slice, [c8 128, (h 2, b 128)]
    #  xcs:  M lhsT for my block, [h 2][b 128, (k 8, c 128)]
    #  xcs8: M lhsT for block 8 (same on all cores)
    #  wblk: W rows of my block, (128, (j 10, d 16, k 8))
    #  w8f:  W rows of block 8 (same on all cores)
    xTs_d = nc.dram_tensor("xTs", [128, 2048], f32r, kind="ExternalInput").ap()
    xT8k_d = nc.dram_tensor("xT8k", [128, 256], f32r, kind="ExternalInput").ap()
    xcs_d = nc.dram_tensor("xcs", [2, 128, 1024], f32r,
                           kind="ExternalInput").ap()
    xcs8_d = nc.dram_tensor("xcs8", [2, 128, 1024], f32r,
                            kind="ExternalInput").ap()
    wblk_d = nc.dram_tensor("wblk", [128, 1280], f32, kind="ExternalInput").ap()
    w8f_d = nc.dram_tensor("w8f", [128, 1280], f32, kind="ExternalInput").ap()
    w8kr_d = nc.dram_tensor("w8kr", [128, JD], f32, kind="ExternalInput").ap()
    vout_d = nc.dram_tensor("vout", [BS, JD], f32, kind="ExternalOutput").ap()

    rg = [list(range(ncores))]

    with tile.TileContext(nc) as tc:
        with (
            tc.tile_pool(name="const", bufs=1) as pc,
            tc.tile_pool(name="work", bufs=3) as pw,
            tc.tile_pool(name="small", bufs=3) as psm,
            tc.tile_pool(name="ps_s", bufs=1, space="PSUM") as pps,
            tc.tile_pool(name="ps_m", bufs=1, space="PSUM") as ppm,
            tc.tile_pool(name="dram", bufs=2, space="DRAM") as pd,
        ):
            mult = mybir.AluOpType.mult
            add = mybir.AluOpType.add

            # ---- warm-up collective: absorb the NEFF-start core skew and
            # CC-stream init into the input-DMA window so the first real
            # AllReduce starts with ~1us trigger delay instead of ~40us.
            wu_in = pd.tile([1, 4], f32, tag="wu_in")
            wu_out = pd.tile([1, 4], f32, tag="wu_out", addr_space="Shared")
            wz = psm.tile([1, 4], f32, tag="wz")
            nc.vector.memset(wz[:], 0.0)
            nc.sync.dma_start(wu_in[:], wz[:])
            if not nocc:
                nc.gpsimd.collective_compute(
                    "AllReduce", add, replica_groups=rg,
                    ins=[wu_in.opt()], outs=[wu_out.opt()])
            else:
                nc.sync.dma_start(wu_out[:], wu_in[:])

            # ---- persistent tiles ----
            wblk = pc.tile([128, 1280], f32, tag="wblk")
            w8f = pc.tile([128, 1280], f32, tag="w8f")
            wcb = pc.tile([128, 2048], f32r, tag="wcb")     # my block's Wc
            wc8 = pc.tile([128, 256], f32r, tag="wc8")      # block-8 k-slice Wc
            xTsr = pc.tile([128, 2048], f32r, tag="xTsr")
            xT8kr = pc.tile([128, 256], f32r, tag="xT8kr")
            xcsr = [pc.tile([128, 1024], f32r, tag=f"xcsr{h}", name=f"xcsr{h}")
                    for h in range(2)]
            xcs8r = [pc.tile([128, 1024], f32r, tag=f"xcs8r{h}",
                             name=f"xcs8r{h}") for h in range(2)]
            vr = pc.tile([128, 512], f32r, tag="vr")        # padded v, 2 halves
            # b_ij for my block (cols 0..9) and block 8 (cols 10..19)
            bij2 = pc.tile([128, 2 * J], f32, tag="bij2")

            w8kr = pc.tile([128, JD], f32, tag="w8kr")
            nc.sync.dma_start(wblk[:], wblk_d)
            nc.sync.dma_start(w8f[:], w8f_d)
            nc.sync.dma_start(w8kr[:], w8kr_d)
            nc.sync.dma_start(xTsr[:], xTs_d)
            nc.sync.dma_start(xT8kr[:], xT8k_d)
            for h in range(2):
                nc.sync.dma_start(xcsr[h][:], xcs_d[h])
                nc.sync.dma_start(xcs8r[h][:], xcs8_d[h])
            zeros = pc.tile([128, 768], f32, tag="zeros")
            nc.vector.memset(zeros[:], 0.0)
            nc.vector.tensor_copy(wcb[:, 1280:2048], zeros[:])
            nc.vector.tensor_copy(wc8[:, 160:256], zeros[:, :96])
            nc.vector.tensor_copy(vr[:, :512], zeros[:, :512])
            nc.vector.memset(bij2[:], 0.0)

            # my k-slice of block-8 W, viewed (j, d)
            w8kv = w8kr[:].rearrange("p (j d) -> p j d", j=J, d=D)

            for it in range(NIT):
                with nc.named_scope(f"iter{it}"):
                    # ---- c_ij -> Wc for my block + block-8 k-slice ----
                    if it == 0:
                        nc.vector.tensor_scalar_mul(
                            wcb[:, :1280], wblk[:], 0.1)
                        nc.vector.tensor_scalar_mul(
                            wc8[:].rearrange(
                                "p (j d) -> p j d", j=16, d=16)[:, :J, :],
                            w8kv, 0.1)
                    else:
                        exps = psm.tile([128, 2 * J], f32, tag="exps")
                        mx = psm.tile([128, 2], f32, tag="mx")
                        sumex = psm.tile([128, 2], f32, tag="sumex")
                        rcp = psm.tile([128, 2], f32, tag="rcp")
                        b2v = bij2[:].rearrange("p (g j) -> p g j", g=2, j=J)
                        nc.vector.tensor_reduce(
                            mx[:], b2v, axis=mybir.AxisListType.X,
                            op=mybir.AluOpType.max)
                        nc.vector.tensor_tensor(
                            exps[:].rearrange("p (g j) -> p g j", g=2, j=J),
                            b2v,
                            mx[:].unsqueeze(2).broadcast_to([128, 2, J]),
                            op=mybir.AluOpType.subtract)
                        nc.scalar.activation(
                            exps[:], exps[:],
                            mybir.ActivationFunctionType.Exp)
                        nc.vector.tensor_reduce(
                            sumex[:],
                            exps[:].rearrange("p (g j) -> p g j", g=2, j=J),
                            axis=mybir.AxisListType.X, op=add)
                        nc.vector.reciprocal(rcp[:], sumex[:])
                        nc.vector.scalar_tensor_tensor(
                            out=wcb[:, :1280].rearrange(
                                "p (j x) -> p j x", j=J, x=128),
                            in0=wblk[:].rearrange(
                                "p (j x) -> p j x", j=J, x=128),
                            scalar=rcp[:, 0:1],
                            in1=exps[:, :J].unsqueeze(2).broadcast_to(
                                [128, J, 128]),
                            op0=mult, op1=mult)
                        nc.vector.scalar_tensor_tensor(
                            out=wc8[:].rearrange(
                                "p (j d) -> p j d", j=16, d=16)[:, :J, :],
                            in0=w8kv, scalar=rcp[:, 1:2],
                            in1=exps[:, J:].unsqueeze(2).broadcast_to(
                                [128, J, D]),
                            op0=mult, op1=mult)

                    # ---- s partial: my block (all k) + block-8 k=r ----
                    ps_s = [pps.tile([128, 256], f32, tag=f"ps_s{h}",
                                     name=f"ps_s{h}") for h in range(2)]
                    for h in range(2):
                        for k in range(K):
                            nc.tensor.matmul(
                                ps_s[h][:],
                                xTsr[:, (k * 2 + h) * 128:(k * 2 + h + 1) * 128],
                                wcb[:].rearrange(
                                    "p (j d k) -> p j d k",
                                    j=16, d=16, k=8)[:, :, :, k],
                                start=(k == 0), stop=False)
                        nc.tensor.matmul(
                            ps_s[h][:], xT8kr[:, h * 128:(h + 1) * 128],
                            wc8[:], start=False, stop=True)

                    # ---- AllReduce s over the 8 c-shards ----
                    ssb = psm.tile([128, 2 * JD], f32, tag="ssb")
                    for h in range(2):
                        nc.vector.tensor_copy(
                            ssb[:, h * JD:(h + 1) * JD].rearrange(
                                "p (j d) -> p j d", j=J, d=D),
                            ps_s[h][:].rearrange(
                                "p (j d) -> p j d", j=16, d=16)[:, :J, :])
                    sb_dram = pd.tile([128, 2, JD], f32, tag="sb_dram")
                    for h in range(2):
                        nc.sync.dma_start(sb_dram[:, h],
                                          ssb[:, h * JD:(h + 1) * JD])

                    if it == NIT - 1:
                        # final iteration: ReduceScatter; this core gets batch
                        # rows {h*128 + 16r + q} as (2q + h, jd)
                        rs_dram = pd.tile([BS, JD], f32, tag="rs_dram")
                        if not nocc:
                            nc.gpsimd.collective_compute(
                                "ReduceScatter", add, replica_groups=rg,
                                ins=[sb_dram.opt()], outs=[rs_dram.opt()])
                        else:
                            nc.sync.dma_start(
                                rs_dram[:].rearrange(
                                    "(q h) x -> q h x", q=16, h=2),
                                sb_dram[0:16])
                        rsb = psm.tile([BS, JD], f32, tag="rsb")
                        nc.sync.dma_start(rsb[:], rs_dram[:])
                        tf = psm.tile([BS, JD], f32, tag="tf")
                        sqf = psm.tile([BS, JD], f32, tag="sqf")
                        msf = psm.tile([BS, J], f32, tag="msf")
                        smf = psm.tile([BS, J], f32, tag="smf")
                        onf = psm.tile([BS, J], f32, tag="onf")
                        rcf = psm.tile([BS, J], f32, tag="rcf")
                        fcf = psm.tile([BS, J], f32, tag="fcf")
                        vf = psm.tile([BS, JD], f32, tag="vf")
                        nc.vector.tensor_scalar_add(tf[:], rsb[:], 1e-5)
                        nc.vector.tensor_tensor(sqf[:], tf[:], tf[:], op=mult)
                        nc.vector.tensor_reduce(
                            msf[:],
                            sqf[:].rearrange("p (j d) -> p j d", j=J, d=D),
                            axis=mybir.AxisListType.X, op=add)
                        nc.scalar.sqrt(smf[:], msf[:])
                        nc.vector.tensor_scalar_add(onf[:], msf[:], 1.0)
                        nc.vector.reciprocal(rcf[:], onf[:])
                        nc.vector.tensor_tensor(fcf[:], smf[:], rcf[:],
                                                op=mult)
                        nc.vector.tensor_tensor(
                            vf[:].rearrange("p (j d) -> p j d", j=J, d=D),
                            tf[:].rearrange("p (j d) -> p j d", j=J, d=D),
                            fcf[:].unsqueeze(2).broadcast_to([BS, J, D]),
                            op=mult)
                        nc.sync.dma_start(vout_d, vf[:])
                        continue

                    sr_dram = pd.tile([128, 2, JD], f32, tag="sr_dram",
                                      addr_space="Shared")
                    if not nocc:
                        nc.gpsimd.collective_compute(
                            "AllReduce", add, replica_groups=rg,
                            ins=[sb_dram.opt()], outs=[sr_dram.opt()])
                    else:
                        nc.sync.dma_start(sr_dram[:], sb_dram[:])
                    ssum = psm.tile([128, 2 * JD], f32, tag="ssum")
                    for h in range(2):
                        nc.sync.dma_start(ssum[:, h * JD:(h + 1) * JD],
                                          sr_dram[:, h])

                    # ---- squash over the full batch (128 x 2 halves) ----
                    t = psm.tile([128, 2 * JD], f32, tag="t")
                    sq = psm.tile([128, 2 * JD], f32, tag="sq")
                    magsq = psm.tile([128, 2 * J], f32, tag="magsq")
                    sqm = psm.tile([128, 2 * J], f32, tag="sqm")
                    onep = psm.tile([128, 2 * J], f32, tag="onep")
                    rcp2 = psm.tile([128, 2 * J], f32, tag="rcp2")
                    fct = psm.tile([128, 2 * J], f32, tag="fct")
                    v = psm.tile([128, 2 * JD], f32, tag="v")
                    nc.vector.tensor_scalar_add(t[:], ssum[:], 1e-5)
                    nc.vector.tensor_tensor(sq[:], t[:], t[:], op=mult)
                    nc.vector.tensor_reduce(
                        magsq[:],
                        sq[:].rearrange("p (hj d) -> p hj d", hj=2 * J, d=D),
                        axis=mybir.AxisListType.X, op=add)
                    nc.scalar.sqrt(sqm[:], magsq[:])
                    nc.vector.tensor_scalar_add(onep[:], magsq[:], 1.0)
                    nc.vector.reciprocal(rcp2[:], onep[:])
                    nc.vector.tensor_tensor(fct[:], sqm[:], rcp2[:], op=mult)
                    nc.vector.tensor_tensor(
                        v[:].rearrange("p (hj d) -> p hj d", hj=2 * J, d=D),
                        t[:].rearrange("p (hj d) -> p hj d", hj=2 * J, d=D),
                        fct[:].unsqueeze(2).broadcast_to([128, 2 * J, D]),
                        op=mult)

                    # round v into the j-padded f32r tile (2 halves)
                    nc.vector.tensor_copy(
                        vr[:].rearrange("p (h j d) -> p h j d",
                                        h=2, j=16, d=16)[:, :, :J, :],
                        v[:].rearrange("p (h j d) -> p h j d",
                                       h=2, j=J, d=D))

                    # ---- M matmuls: my block then block 8 ----
                    for lhs, m3tag in ((xcsr, "m3b"), (xcs8r, "m38")):
                        ps_m = ppm.tile([128, 2048], f32, tag="ps_m",
                                        name="ps_m")
                        for k in range(K):
                            for h in range(2):
                                nc.tensor.matmul(
                                    ps_m[:, k * 256:(k + 1) * 256],
                                    lhs[h][:, k * 128:(k + 1) * 128],
                                    vr[:, h * 256:(h + 1) * 256],
                                    start=(h == 0), stop=(h == 1))
                        m3 = pw.tile([128, 1280], f32, tag=m3tag, name=m3tag)
                        nc.scalar.copy(
                            m3[:].rearrange("p (j d k) -> p k j d",
                                            j=J, d=D, k=K),
                            ps_m[:].rearrange("p (k j2 d) -> p k j2 d",
                                              k=K, j2=16, d=16)[:, :, :J, :])
                        pt = pw.tile([128, 1280], f32, tag=f"pt{m3tag}",
                                     name=f"pt{m3tag}")
                        wsrc = wblk if m3tag == "m3b" else w8f
                        bt = bij2[:, :J] if m3tag == "m3b" else bij2[:, J:]
                        nc.vector.tensor_tensor(pt[:], wsrc[:], m3[:], op=mult)
                        agr = psm.tile([128, J], f32, tag=f"agr{m3tag}",
                                       name=f"agr{m3tag}")
                        nc.vector.tensor_reduce(
                            agr[:],
                            pt[:].rearrange("p (j x) -> p j x", j=J, x=D * K),
                            axis=mybir.AxisListType.X, op=add)
                        nc.vector.tensor_tensor(bt, bt, agr[:], op=add)

    nc.compile()
    return nc


def _tf32(a):
    """Round fp32 -> tf32 bit pattern (round-to-nearest-even on 13 bits)."""
    u = np.ascontiguousarray(a, np.float32).view(np.uint32)
    r = u + np.uint32(0xFFF) + ((u >> np.uint32(13)) & np.uint32(1))
    return (r & np.uint32(0xFFFFE000)).view(np.float32)


def _prep_inputs(x, W):
    """Host-side shard + relayout (x is pre-rounded to the tf32 grid the
    tensor engine would use anyway)."""
    x = _tf32(np.ascontiguousarray(x, dtype=np.float32))
    W0 = np.ascontiguousarray(W.reshape(C, J, D, K), dtype=np.float32)
    # block-8 tensors (identical on every core)
    x8 = x[:, :, 1024:1152]                                  # (256, 8, 128)
    xcs8 = np.ascontiguousarray(x8.transpose(0, 1, 2)).reshape(2, 128, 1024)
    w8f = W0[1024:1152].reshape(128, 1280)
    in_maps = []
    for r in range(NCORES):
        xb = x[:, :, r * 128:(r + 1) * 128]                  # (256, 8, 128)
        # xTs[c, (k, h, b)] = x[h*128+b, k, cb_r*128+c]
        xTs = np.ascontiguousarray(
            xb.reshape(2, 128, K, 128).transpose(3, 2, 0, 1)).reshape(128, 2048)
        # xT8k[c8, (h, b)] = x[h*128+b, r, 1024+c8]
        xT8k = np.ascontiguousarray(
            x[:, r, 1024:1152].reshape(2, 128, 128).transpose(2, 0, 1)
        ).reshape(128, 256)
        xcs = np.ascontiguousarray(xb).reshape(2, 128, 1024)
        wblk = np.ascontiguousarray(W0[r * 128:(r + 1) * 128].reshape(128, 1280))
        w8kr = np.ascontiguousarray(W0[1024:1152, :, :, r].reshape(128, JD))
        in_maps.append({
            "xTs": xTs, "xT8k": xT8k, "xcs": xcs, "xcs8": xcs8,
            "wblk": wblk, "w8f": w8f, "w8kr": w8kr,
        })
    return in_maps


def kernel(x, W):
    global LAST_RESULTS
    from concourse.bass_utils import run_bass_kernel_spmd

    if "nc" not in _CACHE:
        _CACHE["nc"] = _build()
    nc = _CACHE["nc"]
    in_maps = _prep_inputs(np.asarray(x), np.asarray(W))
    last_err = None
    for attempt in range(3):
        try:
            res = run_bass_kernel_spmd(
                nc, in_maps, core_ids=list(range(NCORES)),
                trace=bool(os.environ.get("CAPS_TRACE")))
            break
        except Exception as e:  # device may need a recovery window
            last_err = e
            import time
            time.sleep(90)
    else:
        raise last_err
    LAST_RESULTS = res
    # core r's vout row (2q + h) holds batch row h*128 + 16r + q
    out = np.empty((B, JD), np.float32)
    for r in range(NCORES):
        vr_ = res.results[r]["vout"].reshape(16, 2, JD)      # (q, h, jd)
        out[16 * r:16 * r + 16] = vr_[:, 0]
        out[128 + 16 * r:128 + 16 * r + 16] = vr_[:, 1]
    return np.ascontiguousarray(out.reshape(B, J, D)[..., None]).astype(
        np.float32)



# revision 7
# speedup vs baseline: 1.0354x; 1.0354x over previous
"""CapsuleLayer dynamic-routing kernel for 8x Trainium2 NeuronCores.

Problem: x (256, 8, 1152) f32, W (1, 1152, 10, 16, 8) f32 ->
v (256, 10, 16, 1) f32 via 4 routing iterations.

u_hat (189 MB) is never materialized; each routing iteration is computed
in factorized form directly from x and W:
    s[b, jd]  = sum_{ck} xT[ck, b] * (c_ij[c, j] * W[c, j, d, k])   (PE)
    M[c, jdk] = sum_b x[b, kc] * v[b, jd]                            (PE)
    agr[c, j] = sum_{dk} W[c, j, d, k] * M[c, j, d, k]               (DVE)

Sharding: the routing state is c-sharded. Core r owns c-block r (128 of
the 1152 input capsules); the 9th block is replicated (every core
computes its full M/agreement so its b_ij stays consistent without a
collective, while its s-contribution is k-striped so the sum is counted
once). b_ij, softmax, Wc, M and the agreement never leave the core; the
only cross-core exchange is ONE AllReduce per iteration of the s partial
sums (256 x 160 fp16, 80 KB), which also serves as the full-batch v
broadcast for the next agreement. W is staged k-major and j-padded to 16
([c, (k, j2, d)]) so it is elementwise-congruent with the M PSUM layout:
the agreement runs directly on PSUM with no layout-fixing copy, and the
per-k Wc slices are contiguous 256-column matmul operands. Matmul
operands are rounded to float32r (TF32) for 4x PE throughput with fp32
PSUM accumulation. End-to-end absmax relative error vs the fp32 oracle
is ~1e-3.
"""
import os
import numpy as np

C, J, D, K = 1152, 10, 16, 8
B, NCORES = 256, 8
BS = B // NCORES
CB = C // 128               # 9 c-blocks; blocks 0..7 sharded, block 8 shared
JD = J * D                  # 160
NIT = 4

_CACHE = {}
LAST_RESULTS = None


def _build(ncores=NCORES, nocc=False):
    """Build + compile the per-core program.

    nocc=True: timing variant for TimelineSim -- collectives are replaced
    by equivalent-size local DMAs that keep the serializing dependency.
    """
    from concourse import bass, bacc, tile, mybir

    f32 = mybir.dt.float32
    f16 = mybir.dt.float16
    f32r = mybir.dt.float32r
    nc = bacc.Bacc("TRN2", target_bir_lowering=False, debug=False,
                   num_devices=ncores)

    # per-core inputs (host-sharded):
    #  xTs:   lhsT for my s-block, [c 128, (k 8, h 2, b 128)]
    #  xT8k:  lhsT for block-8 k=r slice, [c8 128, (h 2, b 128)]
    #  xcs:   M lhsT for my block, [h 2][b 128, (k 8, c 128)]
    #  xcs8:  M lhsT for block 8 (same on all cores)
    #  wpad:  W rows of my block, k-major j-padded (128, (k 8, j2 16, d 16))
    #  w8pad: W rows of block 8, same layout (same on all cores)
    #  w8kr:  my k-slice of block-8 W, (128, (j 10, d 16))
    xTs_d = nc.dram_tensor("xTs", [128, 2048], f32r, kind="ExternalInput").ap()
    xT8k_d = nc.dram_tensor("xT8k", [128, 256], f32r, kind="ExternalInput").ap()
    xcs_d = nc.dram_tensor("xcs", [2, 128, 1024], f32r,
                           kind="ExternalInput").ap()
    xcs8_d = nc.dram_tensor("xcs8", [2, 128, 1024], f32r,
                            kind="ExternalInput").ap()
    wpad_d = nc.dram_tensor("wpad", [128, 2048], f32, kind="ExternalInput").ap()
    w8pad_d = nc.dram_tensor("w8pad", [128, 2048], f32,
                             kind="ExternalInput").ap()
    w8kr_d = nc.dram_tensor("w8kr", [128, JD], f32, kind="ExternalInput").ap()
    vout_d = nc.dram_tensor("vout", [BS, JD], f32, kind="ExternalOutput").ap()

    rg = [list(range(ncores))]

    with tile.TileContext(nc) as tc:
        with (
            tc.tile_pool(name="const", bufs=1) as pc,
            tc.tile_pool(name="work", bufs=3) as pw,
            tc.tile_pool(name="small", bufs=3) as psm,
            tc.tile_pool(name="ps_s", bufs=1, space="PSUM") as pps,
            tc.tile_pool(name="ps_m", bufs=1, space="PSUM") as ppm,
            tc.tile_pool(name="dram", bufs=2, space="DRAM") as pd,
        ):
            mult = mybir.AluOpType.mult
            add = mybir.AluOpType.add

            # ---- persistent tiles ----
            wpad = pc.tile([128, 2048], f32, tag="wpad")
            w8pad = pc.tile([128, 2048], f32, tag="w8pad")
            wcb = pc.tile([128, 2048], f32r, tag="wcb")     # my block's Wc
            wc8 = pc.tile([128, 256], f32r, tag="wc8")      # block-8 k-slice Wc
            xTsr = pc.tile([128, 2048], f32r, tag="xTsr")
            xT8kr = pc.tile([128, 256], f32r, tag="xT8kr")
            xcsr = [pc.tile([128, 1024], f32r, tag=f"xcsr{h}", name=f"xcsr{h}")
                    for h in range(2)]
            xcs8r = [pc.tile([128, 1024], f32r, tag=f"xcs8r{h}",
                             name=f"xcs8r{h}") for h in range(2)]
            vr = pc.tile([128, 512], f32r, tag="vr")        # padded v, 2 halves
            # b_ij for my block (cols 0..9) and block 8 (cols 10..19)
            bij2 = pc.tile([128, 2 * J], f32, tag="bij2")

            w8kr = pc.tile([128, JD], f32, tag="w8kr")
            nc.sync.dma_start(wpad[:], wpad_d)
            nc.sync.dma_start(w8kr[:], w8kr_d)
            nc.sync.dma_start(xTsr[:], xTs_d)
            nc.sync.dma_start(xT8kr[:], xT8k_d)
            for h in range(2):
                nc.sync.dma_start(xcsr[h][:], xcs_d[h])
                nc.sync.dma_start(xcs8r[h][:], xcs8_d[h])
            nc.sync.dma_start(w8pad[:], w8pad_d)
            zeros = pc.tile([128, 512], f32, tag="zeros")
            nc.vector.memset(zeros[:], 0.0)
            nc.vector.tensor_copy(wc8[:, 160:256], zeros[:, :96])
            nc.vector.tensor_copy(vr[:, :512], zeros[:, :512])
            nc.vector.memset(bij2[:], 0.0)

            # my k-slice of block-8 W, viewed (j, d)
            w8kv = w8kr[:].rearrange("p (j d) -> p j d", j=J, d=D)

            for it in range(NIT):
                with nc.named_scope(f"iter{it}"):
                    # ---- c_ij -> Wc for my block + block-8 k-slice ----
                    # wcb pad rows (j2 10..15) are written once from the
                    # zero-padded wpad at it==0 and never touched again.
                    if it == 0:
                        nc.vector.tensor_scalar_mul(wcb[:], wpad[:], 0.1)
                        nc.vector.tensor_scalar_mul(
                            wc8[:].rearrange(
                                "p (j d) -> p j d", j=16, d=16)[:, :J, :],
                            w8kv, 0.1)
                    else:
                        exps = psm.tile([128, 2 * J], f32, tag="exps")
                        mx = psm.tile([128, 2], f32, tag="mx")
                        sumex = psm.tile([128, 2], f32, tag="sumex")
                        rcp = psm.tile([128, 2], f32, tag="rcp")
                        b2v = bij2[:].rearrange("p (g j) -> p g j", g=2, j=J)
                        nc.vector.tensor_reduce(
                            mx[:], b2v, axis=mybir.AxisListType.X,
                            op=mybir.AluOpType.max)
                        nc.vector.tensor_tensor(
                            exps[:].rearrange("p (g j) -> p g j", g=2, j=J),
                            b2v,
                            mx[:].unsqueeze(2).broadcast_to([128, 2, J]),
                            op=mybir.AluOpType.subtract)
                        nc.scalar.activation(
                            exps[:], exps[:],
                            mybir.ActivationFunctionType.Exp)
                        nc.vector.tensor_reduce(
                            sumex[:],
                            exps[:].rearrange("p (g j) -> p g j", g=2, j=J),
                            axis=mybir.AxisListType.X, op=add)
                        nc.vector.reciprocal(rcp[:], sumex[:])
                        # per-k scale: the first slice unblocks the first s
                        # matmul while the rest of the scaling still runs
                        for k in range(K):
                            nc.vector.scalar_tensor_tensor(
                                out=wcb[:, k * 256:(k + 1) * 256].rearrange(
                                    "p (j d) -> p j d", j=16, d=16)[:, :J, :],
                                in0=wpad[:, k * 256:(k + 1) * 256].rearrange(
                                    "p (j d) -> p j d", j=16, d=16)[:, :J, :],
                                scalar=rcp[:, 0:1],
                                in1=exps[:, :J].unsqueeze(2).broadcast_to(
                                    [128, J, D]),
                                op0=mult, op1=mult)
                        nc.vector.scalar_tensor_tensor(
                            out=wc8[:].rearrange(
                                "p (j d) -> p j d", j=16, d=16)[:, :J, :],
                            in0=w8kv, scalar=rcp[:, 1:2],
                            in1=exps[:, J:].unsqueeze(2).broadcast_to(
                                [128, J, D]),
                            op0=mult, op1=mult)

                    # ---- s partial: my block (all k) + block-8 k=r ----
                    ps_s = [pps.tile([128, 256], f32, tag=f"ps_s{h}",
                                     name=f"ps_s{h}") for h in range(2)]
                    for h in range(2):
                        for k in range(K):
                            nc.tensor.matmul(
                                ps_s[h][:],
                                xTsr[:, (k * 2 + h) * 128:(k * 2 + h + 1) * 128],
                                wcb[:, k * 256:(k + 1) * 256],
                                start=(k == 0), stop=False)
                        nc.tensor.matmul(
                            ps_s[h][:], xT8kr[:, h * 128:(h + 1) * 128],
                            wc8[:], start=False, stop=True)

                    # ---- AllReduce s over the 8 c-shards (fp16 payload) ----
                    ssb = psm.tile([128, 2 * JD], f16, tag="ssb")
                    for h in range(2):
                        nc.vector.tensor_copy(
                            ssb[:, h * JD:(h + 1) * JD].rearrange(
                                "p (j d) -> p j d", j=J, d=D),
                            ps_s[h][:].rearrange(
                                "p (j d) -> p j d", j=16, d=16)[:, :J, :])
                    sb_dram = pd.tile([128, 2, JD], f16, tag="sb_dram")
                    for h in range(2):
                        nc.sync.dma_start(sb_dram[:, h],
                                          ssb[:, h * JD:(h + 1) * JD])

                    if it == NIT - 1:
                        # final iteration: ReduceScatter; this core gets batch
                        # rows {h*128 + 16r + q} as (2q + h, jd)
                        rs_dram = pd.tile([BS, JD], f16, tag="rs_dram")
                        if not nocc:
                            nc.gpsimd.collective_compute(
                                "ReduceScatter", add, replica_groups=rg,
                                ins=[sb_dram.opt()], outs=[rs_dram.opt()])
                        else:
                            nc.sync.dma_start(
                                rs_dram[:].rearrange(
                                    "(q h) x -> q h x", q=16, h=2),
                                sb_dram[0:16])
                        rsb = psm.tile([BS, JD], f16, tag="rsb")
                        nc.sync.dma_start(rsb[:], rs_dram[:])
                        tf = psm.tile([BS, JD], f32, tag="tf")
                        sqf = psm.tile([BS, JD], f32, tag="sqf")
                        msf = psm.tile([BS, J], f32, tag="msf")
                        smf = psm.tile([BS, J], f32, tag="smf")
                        onf = psm.tile([BS, J], f32, tag="onf")
                        rcf = psm.tile([BS, J], f32, tag="rcf")
                        fcf = psm.tile([BS, J], f32, tag="fcf")
                        vf = psm.tile([BS, JD], f32, tag="vf")
                        nc.vector.tensor_scalar_add(tf[:], rsb[:], 1e-5)
                        nc.vector.tensor_tensor(sqf[:], tf[:], tf[:], op=mult)
                        nc.vector.tensor_reduce(
                            msf[:],
                            sqf[:].rearrange("p (j d) -> p j d", j=J, d=D),
                            axis=mybir.AxisListType.X, op=add)
                        nc.scalar.sqrt(smf[:], msf[:])
                        nc.vector.tensor_scalar_add(onf[:], msf[:], 1.0)
                        nc.vector.reciprocal(rcf[:], onf[:])
                        nc.vector.tensor_tensor(fcf[:], smf[:], rcf[:],
                                                op=mult)
                        nc.vector.tensor_tensor(
                            vf[:].rearrange("p (j d) -> p j d", j=J, d=D),
                            tf[:].rearrange("p (j d) -> p j d", j=J, d=D),
                            fcf[:].unsqueeze(2).broadcast_to([BS, J, D]),
                            op=mult)
                        nc.sync.dma_start(vout_d, vf[:])
                        continue

                    sr_dram = pd.tile([128, 2, JD], f16, tag="sr_dram",
                                      addr_space="Shared")
                    if not nocc:
                        nc.gpsimd.collective_compute(
                            "AllReduce", add, replica_groups=rg,
                            ins=[sb_dram.opt()], outs=[sr_dram.opt()])
                    else:
                        nc.sync.dma_start(sr_dram[:], sb_dram[:])
                    ssum = psm.tile([128, 2 * JD], f16, tag="ssum")
                    for h in range(2):
                        nc.sync.dma_start(ssum[:, h * JD:(h + 1) * JD],
                                          sr_dram[:, h])

                    # ---- squash over the full batch (128 x 2 halves) ----
                    t = psm.tile([128, 2 * JD], f32, tag="t")
                    sq = psm.tile([128, 2 * JD], f32, tag="sq")
                    magsq = psm.tile([128, 2 * J], f32, tag="magsq")
                    sqm = psm.tile([128, 2 * J], f32, tag="sqm")
                    onep = psm.tile([128, 2 * J], f32, tag="onep")
                    rcp2 = psm.tile([128, 2 * J], f32, tag="rcp2")
                    fct = psm.tile([128, 2 * J], f32, tag="fct")
                    v = psm.tile([128, 2 * JD], f32, tag="v")
                    nc.vector.tensor_scalar_add(t[:], ssum[:], 1e-5)
                    nc.vector.tensor_tensor(sq[:], t[:], t[:], op=mult)
                    nc.vector.tensor_reduce(
                        magsq[:],
                        sq[:].rearrange("p (hj d) -> p hj d", hj=2 * J, d=D),
                        axis=mybir.AxisListType.X, op=add)
                    nc.scalar.sqrt(sqm[:], magsq[:])
                    nc.vector.tensor_scalar_add(onep[:], magsq[:], 1.0)
                    nc.vector.reciprocal(rcp2[:], onep[:])
                    nc.vector.tensor_tensor(fct[:], sqm[:], rcp2[:], op=mult)
                    nc.vector.tensor_tensor(
                        v[:].rearrange("p (hj d) -> p hj d", hj=2 * J, d=D),
                        t[:].rearrange("p (hj d) -> p hj d", hj=2 * J, d=D),
                        fct[:].unsqueeze(2).broadcast_to([128, 2 * J, D]),
                        op=mult)

                    # round v into the j-padded f32r tile (2 halves)
                    nc.vector.tensor_copy(
                        vr[:].rearrange("p (h j d) -> p h j d",
                                        h=2, j=16, d=16)[:, :, :J, :],
                        v[:].rearrange("p (h j d) -> p h j d",
                                       h=2, j=J, d=D))

                    # ---- M matmuls + PSUM-direct agreement per block ----
                    # ps_m layout (k, j2 16, d) matches wpad/w8pad exactly,
                    # so the agreement is an elementwise product on PSUM, a
                    # d-reduce, and a 3-step k-tree -- no layout-fixing copy.
                    for lhs, wsrc, bcol, mtag in (
                            (xcsr, wpad, 0, "mb"), (xcs8r, w8pad, 1, "m8")):
                        ps_m = ppm.tile([128, 2048], f32, tag="ps_m",
                                        name=f"ps_m_{mtag}")
                        for k in range(K):
                            for h in range(2):
                                nc.tensor.matmul(
                                    ps_m[:, k * 256:(k + 1) * 256],
                                    lhs[h][:, k * 128:(k + 1) * 128],
                                    vr[:, h * 256:(h + 1) * 256],
                                    start=(h == 0), stop=(h == 1))
                        pt = pw.tile([128, 2048], f32, tag=f"pt{mtag}",
                                     name=f"pt{mtag}")
                        nc.vector.tensor_tensor(pt[:], wsrc[:], ps_m[:],
                                                op=mult)
                        kj = psm.tile([128, 128], f32, tag=f"kj{mtag}",
                                      name=f"kj{mtag}")
                        nc.vector.tensor_reduce(
                            kj[:],
                            pt[:].rearrange("p (kj d) -> p kj d",
                                            kj=128, d=16),
                            axis=mybir.AxisListType.X, op=add)
                        t64 = psm.tile([128, 64], f32, tag=f"t64{mtag}",
                                       name=f"t64{mtag}")
                        t32 = psm.tile([128, 32], f32, tag=f"t32{mtag}",
                                       name=f"t32{mtag}")
                        a16 = psm.tile([128, 16], f32, tag=f"a16{mtag}",
                                       name=f"a16{mtag}")
                        nc.vector.tensor_tensor(t64[:], kj[:, :64],
                                                kj[:, 64:], op=add)
                        nc.vector.tensor_tensor(t32[:], t64[:, :32],
                                                t64[:, 32:], op=add)
                        nc.vector.tensor_tensor(a16[:], t32[:, :16],
                                                t32[:, 16:], op=add)
                        bt = bij2[:, bcol * J:(bcol + 1) * J]
                        nc.vector.tensor_tensor(bt, bt, a16[:, :J], op=add)

    nc.compile()
    return nc


def _tf32(a):
    """Round fp32 -> tf32 bit pattern (round-to-nearest-even on 13 bits)."""
    u = np.ascontiguousarray(a, np.float32).view(np.uint32)
    r = u + np.uint32(0xFFF) + ((u >> np.uint32(13)) & np.uint32(1))
    return (r & np.uint32(0xFFFFE000)).view(np.float32)


def _pad_w(wrows):
    """(128, J, D, K) W rows -> k-major j2=16-padded (128, 2048)."""
    wp = np.zeros((128, K, 16, D), np.float32)
    wp[:, :, :J, :] = wrows.transpose(0, 3, 1, 2)
    return np.ascontiguousarray(wp).reshape(128, 2048)


def _prep_inputs(x, W):
    """Host-side shard + relayout (x is pre-rounded to the tf32 grid the
    tensor engine would use anyway)."""
    x = _tf32(np.ascontiguousarray(x, dtype=np.float32))
    W0 = np.ascontiguousarray(W.reshape(C, J, D, K), dtype=np.float32)
    # block-8 tensors (identical on every core)
    x8 = x[:, :, 1024:1152]                                  # (256, 8, 128)
    xcs8 = np.ascontiguousarray(x8.transpose(0, 1, 2)).reshape(2, 128, 1024)
    w8pad = _pad_w(W0[1024:1152])
    in_maps = []
    for r in range(NCORES):
        xb = x[:, :, r * 128:(r + 1) * 128]                  # (256, 8, 128)
        # xTs[c, (k, h, b)] = x[h*128+b, k, cb_r*128+c]
        xTs = np.ascontiguousarray(
            xb.reshape(2, 128, K, 128).transpose(3, 2, 0, 1)).reshape(128, 2048)
        # xT8k[c8, (h, b)] = x[h*128+b, r, 1024+c8]
        xT8k = np.ascontiguousarray(
            x[:, r, 1024:1152].reshape(2, 128, 128).transpose(2, 0, 1)
        ).reshape(128, 256)
        xcs = np.ascontiguousarray(xb).reshape(2, 128, 1024)
        wpad = _pad_w(W0[r * 128:(r + 1) * 128])
        w8kr = np.ascontiguousarray(W0[1024:1152, :, :, r].reshape(128, JD))
        in_maps.append({
            "xTs": xTs, "xT8k": xT8k, "xcs": xcs, "xcs8": xcs8,
            "wpad": wpad, "w8pad": w8pad, "w8kr": w8kr,
        })
    return in_maps


def kernel(x, W):
    global LAST_RESULTS
    from concourse.bass_utils import run_bass_kernel_spmd

    if "nc" not in _CACHE:
        _CACHE["nc"] = _build()
    nc = _CACHE["nc"]
    in_maps = _prep_inputs(np.asarray(x), np.asarray(W))
    last_err = None
    for attempt in range(3):
        try:
            res = run_bass_kernel_spmd(
                nc, in_maps, core_ids=list(range(NCORES)),
                trace=bool(os.environ.get("CAPS_TRACE")))
            break
        except Exception as e:  # device may need a recovery window
            last_err = e
            import time
            time.sleep(90)
    else:
        raise last_err
    LAST_RESULTS = res
    # core r's vout row (2q + h) holds batch row h*128 + 16r + q
    out = np.empty((B, JD), np.float32)
    for r in range(NCORES):
        vr_ = res.results[r]["vout"].reshape(16, 2, JD)      # (q, h, jd)
        out[16 * r:16 * r + 16] = vr_[:, 0]
        out[128 + 16 * r:128 + 16 * r + 16] = vr_[:, 1]
    return np.ascontiguousarray(out.reshape(B, J, D)[..., None]).astype(
        np.float32)


# revision 9
# speedup vs baseline: 1.1096x; 1.0717x over previous
"""CapsuleLayer dynamic-routing kernel for 8x Trainium2 NeuronCores.

Problem: x (256, 8, 1152) f32, W (1, 1152, 10, 16, 8) f32 ->
v (256, 10, 16, 1) f32 via 4 routing iterations.

u_hat (189 MB) is never materialized; each routing iteration is computed
in factorized form directly from x and W:
    s[b, jd]  = sum_{ck} xT[ck, b] * (c_ij[c, j] * W[c, j, d, k])   (PE)
    M[c, jdk] = sum_b x[b, kc] * v[b, jd]                            (PE)
    agr[c, j] = sum_{dk} W[c, j, d, k] * M[c, j, d, k]               (DVE)

Sharding: the routing state is c-sharded. Core r owns c-block r (128 of
the 1152 input capsules); the 9th block is replicated (every core
computes its full M/agreement so its b_ij stays consistent without a
collective, while its s-contribution is k-striped so the sum is counted
once). b_ij, softmax, Wc, M and the agreement never leave the core; the
only cross-core exchange is ONE AllReduce per iteration of the s partial
sums (256 x 160 fp16, 80 KB), which also serves as the full-batch v
broadcast for the next agreement. W is staged k-major and j-padded to 16
([c, (k, j2, d)]) so it is elementwise-congruent with the M PSUM layout:
the agreement runs directly on PSUM with no layout-fixing copy, and the
per-k Wc slices are contiguous 256-column matmul operands. Matmul
operands are rounded to float32r (TF32) for 4x PE throughput with fp32
PSUM accumulation. End-to-end absmax relative error vs the fp32 oracle
is ~1e-3.
"""
import os
import numpy as np

C, J, D, K = 1152, 10, 16, 8
B, NCORES = 256, 8
BS = B // NCORES
CB = C // 128               # 9 c-blocks; blocks 0..7 sharded, block 8 shared
JD = J * D                  # 160
NIT = 4

_CACHE = {}
LAST_RESULTS = None


def _build(ncores=NCORES, nocc=False):
    """Build + compile the per-core program.

    nocc=True: timing variant for TimelineSim -- collectives are replaced
    by equivalent-size local DMAs that keep the serializing dependency.
    """
    from concourse import bass, bacc, tile, mybir

    f32 = mybir.dt.float32
    f16 = mybir.dt.float16
    f32r = mybir.dt.float32r
    nc = bacc.Bacc("TRN2", target_bir_lowering=False, debug=False,
                   num_devices=ncores)

    # per-core inputs (host-sharded):
    #  xTs:   lhsT for my s-block, [c 128, (k 8, h 2, b 128)]
    #  xT8k:  lhsT for block-8 k=r slice, [c8 128, (h 2, b 128)]
    #  xcs:   M lhsT for my block, [h 2][b 128, (k 8, c 128)]
    #  xcs8:  M lhsT for block 8 (same on all cores)
    #  wpad:  W rows of my block, k-major j-padded (128, (k 8, j2 16, d 16))
    #  w8pad: W rows of block 8, same layout (same on all cores)
    #  w8kr:  my k-slice of block-8 W, (128, (j 10, d 16))
    xTs_d = nc.dram_tensor("xTs", [128, 2048], f32r, kind="ExternalInput").ap()
    xT8k_d = nc.dram_tensor("xT8k", [128, 256], f32r, kind="ExternalInput").ap()
    xcs_d = nc.dram_tensor("xcs", [2, 128, 1024], f32r,
                           kind="ExternalInput").ap()
    xcs8_d = nc.dram_tensor("xcs8", [2, 128, 1024], f32r,
                            kind="ExternalInput").ap()
    wpad_d = nc.dram_tensor("wpad", [128, 2048], f32, kind="ExternalInput").ap()
    w8pad_d = nc.dram_tensor("w8pad", [128, 2048], f32,
                             kind="ExternalInput").ap()
    w8kr_d = nc.dram_tensor("w8kr", [128, JD], f32, kind="ExternalInput").ap()
    vout_d = nc.dram_tensor("vout", [BS, JD], f32, kind="ExternalOutput").ap()

    rg = [list(range(ncores))]

    with tile.TileContext(nc) as tc:
        with (
            tc.tile_pool(name="const", bufs=1) as pc,
            tc.tile_pool(name="work", bufs=3) as pw,
            tc.tile_pool(name="small", bufs=3) as psm,
            tc.tile_pool(name="ps_s", bufs=1, space="PSUM") as pps,
            tc.tile_pool(name="ps_m", bufs=1, space="PSUM") as ppm,
            tc.tile_pool(name="dram", bufs=2, space="DRAM") as pd,
        ):
            mult = mybir.AluOpType.mult
            add = mybir.AluOpType.add

            # ---- persistent tiles ----
            wpad = pc.tile([128, 2048], f32, tag="wpad")
            w8pad = pc.tile([128, 2048], f32, tag="w8pad")
            wcb = pc.tile([128, 2048], f32r, tag="wcb")     # my block's Wc
            wc8 = pc.tile([128, 256], f32r, tag="wc8")      # block-8 k-slice Wc
            xTsr = pc.tile([128, 2048], f32r, tag="xTsr")
            xT8kr = pc.tile([128, 256], f32r, tag="xT8kr")
            xcsr = [pc.tile([128, 1024], f32r, tag=f"xcsr{h}", name=f"xcsr{h}")
                    for h in range(2)]
            xcs8r = [pc.tile([128, 1024], f32r, tag=f"xcs8r{h}",
                             name=f"xcs8r{h}") for h in range(2)]
            vr = pc.tile([128, 512], f32r, tag="vr")        # padded v, 2 halves
            # b_ij for my block (cols 0..9) and block 8 (cols 10..19)
            bij2 = pc.tile([128, 2 * J], f32, tag="bij2")

            w8kr = pc.tile([128, JD], f32, tag="w8kr")
            nc.sync.dma_start(wpad[:], wpad_d)
            nc.sync.dma_start(w8kr[:], w8kr_d)
            nc.sync.dma_start(xTsr[:], xTs_d)
            nc.sync.dma_start(xT8kr[:], xT8k_d)
            for h in range(2):
                nc.sync.dma_start(xcsr[h][:], xcs_d[h])
                nc.sync.dma_start(xcs8r[h][:], xcs8_d[h])
            nc.sync.dma_start(w8pad[:], w8pad_d)
            zeros = pc.tile([128, 512], f32, tag="zeros")
            nc.vector.memset(zeros[:], 0.0)
            nc.vector.tensor_copy(wc8[:, 160:256], zeros[:, :96])
            nc.vector.tensor_copy(vr[:, :512], zeros[:, :512])
            nc.vector.memset(bij2[:], 0.0)

            # my k-slice of block-8 W, viewed (j, d)
            w8kv = w8kr[:].rearrange("p (j d) -> p j d", j=J, d=D)

            for it in range(NIT):
                with nc.named_scope(f"iter{it}"):
                    # ---- c_ij -> Wc for my block + block-8 k-slice ----
                    # wcb pad rows (j2 10..15) are written once from the
                    # zero-padded wpad at it==0 and never touched again.
                    if it == 0:
                        nc.vector.tensor_scalar_mul(wcb[:], wpad[:], 0.1)
                        nc.vector.tensor_scalar_mul(
                            wc8[:].rearrange(
                                "p (j d) -> p j d", j=16, d=16)[:, :J, :],
                            w8kv, 0.1)
                    else:
                        exps = psm.tile([128, 2 * J], f32, tag="exps")
                        mx = psm.tile([128, 2], f32, tag="mx")
                        sumex = psm.tile([128, 2], f32, tag="sumex")
                        rcp = psm.tile([128, 2], f32, tag="rcp")
                        b2v = bij2[:].rearrange("p (g j) -> p g j", g=2, j=J)
                        nc.vector.tensor_reduce(
                            mx[:], b2v, axis=mybir.AxisListType.X,
                            op=mybir.AluOpType.max)
                        nc.vector.tensor_tensor(
                            exps[:].rearrange("p (g j) -> p g j", g=2, j=J),
                            b2v,
                            mx[:].unsqueeze(2).broadcast_to([128, 2, J]),
                            op=mybir.AluOpType.subtract)
                        nc.scalar.activation(
                            exps[:], exps[:],
                            mybir.ActivationFunctionType.Exp)
                        nc.vector.tensor_reduce(
                            sumex[:],
                            exps[:].rearrange("p (g j) -> p g j", g=2, j=J),
                            axis=mybir.AxisListType.X, op=add)
                        nc.vector.reciprocal(rcp[:], sumex[:])
                        # per-k scale: the first slice unblocks the first s
                        # matmul while the rest of the scaling still runs
                        for k in range(K):
                            nc.vector.scalar_tensor_tensor(
                                out=wcb[:, k * 256:(k + 1) * 256].rearrange(
                                    "p (j d) -> p j d", j=16, d=16)[:, :J, :],
                                in0=wpad[:, k * 256:(k + 1) * 256].rearrange(
                                    "p (j d) -> p j d", j=16, d=16)[:, :J, :],
                                scalar=rcp[:, 0:1],
                                in1=exps[:, :J].unsqueeze(2).broadcast_to(
                                    [128, J, D]),
                                op0=mult, op1=mult)
                        nc.vector.scalar_tensor_tensor(
                            out=wc8[:].rearrange(
                                "p (j d) -> p j d", j=16, d=16)[:, :J, :],
                            in0=w8kv, scalar=rcp[:, 1:2],
                            in1=exps[:, J:].unsqueeze(2).broadcast_to(
                                [128, J, D]),
                            op0=mult, op1=mult)

                    # ---- s partial: my block (all k) + block-8 k=r ----
                    ps_s = [pps.tile([128, 256], f32, tag=f"ps_s{h}",
                                     name=f"ps_s{h}") for h in range(2)]
                    for h in range(2):
                        for k in range(K):
                            nc.tensor.matmul(
                                ps_s[h][:],
                                xTsr[:, (k * 2 + h) * 128:(k * 2 + h + 1) * 128],
                                wcb[:, k * 256:(k + 1) * 256],
                                start=(k == 0), stop=False)
                        nc.tensor.matmul(
                            ps_s[h][:], xT8kr[:, h * 128:(h + 1) * 128],
                            wc8[:], start=False, stop=True)

                    # ---- AllReduce s over the 8 c-shards (fp16 payload) ----
                    ssb = psm.tile([128, 2 * JD], f16, tag="ssb")
                    for h in range(2):
                        nc.vector.tensor_copy(
                            ssb[:, h * JD:(h + 1) * JD].rearrange(
                                "p (j d) -> p j d", j=J, d=D),
                            ps_s[h][:].rearrange(
                                "p (j d) -> p j d", j=16, d=16)[:, :J, :])
                    sb_dram = pd.tile([128, 2, JD], f16, tag="sb_dram")
                    for h in range(2):
                        nc.sync.dma_start(sb_dram[:, h],
                                          ssb[:, h * JD:(h + 1) * JD])

                    if it == NIT - 1:
                        # final iteration: ReduceScatter; this core gets batch
                        # rows {h*128 + 16r + q} as (2q + h, jd)
                        rs_dram = pd.tile([BS, JD], f16, tag="rs_dram")
                        if not nocc:
                            nc.gpsimd.collective_compute(
                                "ReduceScatter", add, replica_groups=rg,
                                ins=[sb_dram.opt()], outs=[rs_dram.opt()])
                        else:
                            nc.sync.dma_start(
                                rs_dram[:].rearrange(
                                    "(q h) x -> q h x", q=16, h=2),
                                sb_dram[0:16])
                        rsb = psm.tile([BS, JD], f16, tag="rsb")
                        nc.sync.dma_start(rsb[:], rs_dram[:])
                        tf = psm.tile([BS, JD], f32, tag="tf")
                        sqf = psm.tile([BS, JD], f32, tag="sqf")
                        msf = psm.tile([BS, J], f32, tag="msf")
                        smf = psm.tile([BS, J], f32, tag="smf")
                        onf = psm.tile([BS, J], f32, tag="onf")
                        rcf = psm.tile([BS, J], f32, tag="rcf")
                        fcf = psm.tile([BS, J], f32, tag="fcf")
                        vf = psm.tile([BS, JD], f32, tag="vf")
                        nc.vector.tensor_scalar_add(tf[:], rsb[:], 1e-5)
                        nc.vector.tensor_tensor(sqf[:], tf[:], tf[:], op=mult)
                        nc.vector.tensor_reduce(
                            msf[:],
                            sqf[:].rearrange("p (j d) -> p j d", j=J, d=D),
                            axis=mybir.AxisListType.X, op=add)
                        nc.scalar.sqrt(smf[:], msf[:])
                        nc.vector.tensor_scalar_add(onf[:], msf[:], 1.0)
                        nc.vector.reciprocal(rcf[:], onf[:])
                        nc.vector.tensor_tensor(fcf[:], smf[:], rcf[:],
                                                op=mult)
                        nc.vector.tensor_tensor(
                            vf[:].rearrange("p (j d) -> p j d", j=J, d=D),
                            tf[:].rearrange("p (j d) -> p j d", j=J, d=D),
                            fcf[:].unsqueeze(2).broadcast_to([BS, J, D]),
                            op=mult)
                        nc.sync.dma_start(vout_d, vf[:])
                        continue

                    sr_dram = pd.tile([128, 2, JD], f16, tag="sr_dram",
                                      addr_space="Shared")
                    if not nocc:
                        nc.gpsimd.collective_compute(
                            "AllReduce", add, replica_groups=rg,
                            ins=[sb_dram.opt()], outs=[sr_dram.opt()])
                    else:
                        nc.sync.dma_start(sr_dram[:], sb_dram[:])
                    ssum = psm.tile([128, 2 * JD], f16, tag="ssum")
                    for h in range(2):
                        nc.sync.dma_start(ssum[:, h * JD:(h + 1) * JD],
                                          sr_dram[:, h])

                    # ---- squash over the full batch (128 x 2 halves) ----
                    # leading ops split per half so half 0 starts as soon as
                    # its DMA-completion semaphore fires
                    t = psm.tile([128, 2 * JD], f32, tag="t")
                    sq = psm.tile([128, 2 * JD], f32, tag="sq")
                    magsq = psm.tile([128, 2 * J], f32, tag="magsq")
                    sqm = psm.tile([128, 2 * J], f32, tag="sqm")
                    onep = psm.tile([128, 2 * J], f32, tag="onep")
                    rcp2 = psm.tile([128, 2 * J], f32, tag="rcp2")
                    fct = psm.tile([128, 2 * J], f32, tag="fct")
                    v = psm.tile([128, 2 * JD], f32, tag="v")
                    for h in range(2):
                        hs = slice(h * JD, (h + 1) * JD)
                        nc.vector.tensor_scalar_add(t[:, hs], ssum[:, hs],
                                                    1e-5)
                        nc.vector.tensor_tensor(sq[:, hs], t[:, hs],
                                                t[:, hs], op=mult)
                        nc.vector.tensor_reduce(
                            magsq[:, h * J:(h + 1) * J],
                            sq[:, hs].rearrange("p (j d) -> p j d",
                                                j=J, d=D),
                            axis=mybir.AxisListType.X, op=add)
                    nc.scalar.sqrt(sqm[:], magsq[:])
                    nc.vector.tensor_scalar_add(onep[:], magsq[:], 1.0)
                    nc.vector.reciprocal(rcp2[:], onep[:])
                    nc.vector.tensor_tensor(fct[:], sqm[:], rcp2[:], op=mult)
                    nc.vector.tensor_tensor(
                        v[:].rearrange("p (hj d) -> p hj d", hj=2 * J, d=D),
                        t[:].rearrange("p (hj d) -> p hj d", hj=2 * J, d=D),
                        fct[:].unsqueeze(2).broadcast_to([128, 2 * J, D]),
                        op=mult)

                    # round v into the j-padded f32r tile (2 halves)
                    nc.vector.tensor_copy(
                        vr[:].rearrange("p (h j d) -> p h j d",
                                        h=2, j=16, d=16)[:, :, :J, :],
                        v[:].rearrange("p (h j d) -> p h j d",
                                       h=2, j=J, d=D))

                    # ---- M matmuls + PSUM-direct agreement per block ----
                    # ps_m layout (k, j2 16, d) matches wpad/w8pad exactly,
                    # so the agreement is an elementwise product on PSUM, a
                    # d-reduce, and a 3-step k-tree -- no layout-fixing copy.
                    for lhs, wsrc, bcol, mtag in (
                            (xcsr, wpad, 0, "mb"), (xcs8r, w8pad, 1, "m8")):
                        ps_m = ppm.tile([128, 2048], f32, tag="ps_m",
                                        name=f"ps_m_{mtag}")
                        for k in range(K):
                            for h in range(2):
                                nc.tensor.matmul(
                                    ps_m[:, k * 256:(k + 1) * 256],
                                    lhs[h][:, k * 128:(k + 1) * 128],
                                    vr[:, h * 256:(h + 1) * 256],
                                    start=(h == 0), stop=(h == 1))
                        # strided product/reduce over the 10 real j rows
                        # only, split into k-halves so the first half's DVE
                        # ops overlap the second half's matmuls (and the WAR
                        # gap before the next block's matmuls shrinks)
                        pt = pw.tile([128, 2 * 640], f32, tag=f"pt{mtag}",
                                     name=f"pt{mtag}")
                        kj = psm.tile([128, 80], f32, tag=f"kj{mtag}",
                                      name=f"kj{mtag}")
                        for g in range(2):
                            gs = slice(g * 1024, (g + 1) * 1024)
                            nc.vector.tensor_tensor(
                                pt[:, g * 640:(g + 1) * 640].rearrange(
                                    "p (k j d) -> p k j d", k=4, j=J, d=D),
                                wsrc[:, gs].rearrange(
                                    "p (k j2 d) -> p k j2 d",
                                    k=4, j2=16, d=16)[:, :, :J, :],
                                ps_m[:, gs].rearrange(
                                    "p (k j2 d) -> p k j2 d",
                                    k=4, j2=16, d=16)[:, :, :J, :],
                                op=mult)
                            nc.vector.tensor_reduce(
                                kj[:, g * 40:(g + 1) * 40],
                                pt[:, g * 640:(g + 1) * 640].rearrange(
                                    "p (kj d) -> p kj d", kj=40, d=16),
                                axis=mybir.AxisListType.X, op=add)
                        t40 = psm.tile([128, 40], f32, tag=f"t40{mtag}",
                                       name=f"t40{mtag}")
                        t20 = psm.tile([128, 20], f32, tag=f"t20{mtag}",
                                       name=f"t20{mtag}")
                        a10 = psm.tile([128, J], f32, tag=f"a10{mtag}",
                                       name=f"a10{mtag}")
                        nc.vector.tensor_tensor(t40[:], kj[:, :40],
                                                kj[:, 40:], op=add)
                        nc.vector.tensor_tensor(t20[:], t40[:, :20],
                                                t40[:, 20:], op=add)
                        nc.vector.tensor_tensor(a10[:], t20[:, :J],
                                                t20[:, J:], op=add)
                        bt = bij2[:, bcol * J:(bcol + 1) * J]
                        nc.vector.tensor_tensor(bt, bt, a10[:], op=add)

    nc.compile()
    return nc


def _tf32(a):
    """Round fp32 -> tf32 bit pattern (round-to-nearest-even on 13 bits)."""
    u = np.ascontiguousarray(a, np.float32).view(np.uint32)
    r = u + np.uint32(0xFFF) + ((u >> np.uint32(13)) & np.uint32(1))
    return (r & np.uint32(0xFFFFE000)).view(np.float32)


def _pad_w(wrows):
    """(128, J, D, K) W rows -> k-major j2=16-padded (128, 2048)."""
    wp = np.zeros((128, K, 16, D), np.float32)
    wp[:, :, :J, :] = wrows.transpose(0, 3, 1, 2)
    return np.ascontiguousarray(wp).reshape(128, 2048)


def _prep_inputs(x, W):
    """Host-side shard + relayout (x is pre-rounded to the tf32 grid the
    tensor engine would use anyway)."""
    x = _tf32(np.ascontiguousarray(x, dtype=np.float32))
    W0 = np.ascontiguousarray(W.reshape(C, J, D, K), dtype=np.float32)
    # block-8 tensors (identical on every core)
    x8 = x[:, :, 1024:1152]                                  # (256, 8, 128)
    xcs8 = np.ascontiguousarray(x8.transpose(0, 1, 2)).reshape(2, 128, 1024)
    w8pad = _pad_w(W0[1024:1152])
    in_maps = []
    for r in range(NCORES):
        xb = x[:, :, r * 128:(r + 1) * 128]                  # (256, 8, 128)
        # xTs[c, (k, h, b)] = x[h*128+b, k, cb_r*128+c]
        xTs = np.ascontiguousarray(
            xb.reshape(2, 128, K, 128).transpose(3, 2, 0, 1)).reshape(128, 2048)
        # xT8k[c8, (h, b)] = x[h*128+b, r, 1024+c8]
        xT8k = np.ascontiguousarray(
            x[:, r, 1024:1152].reshape(2, 128, 128).transpose(2, 0, 1)
        ).reshape(128, 256)
        xcs = np.ascontiguousarray(xb).reshape(2, 128, 1024)
        wpad = _pad_w(W0[r * 128:(r + 1) * 128])
        w8kr = np.ascontiguousarray(W0[1024:1152, :, :, r].reshape(128, JD))
        in_maps.append({
            "xTs": xTs, "xT8k": xT8k, "xcs": xcs, "xcs8": xcs8,
            "wpad": wpad, "w8pad": w8pad, "w8kr": w8kr,
        })
    return in_maps


def kernel(x, W):
    global LAST_RESULTS
    from concourse.bass_utils import run_bass_kernel_spmd

    if "nc" not in _CACHE:
        _CACHE["nc"] = _build()
    nc = _CACHE["nc"]
    in_maps = _prep_inputs(np.asarray(x), np.asarray(W))
    last_err = None
    for attempt in range(3):
        try:
            res = run_bass_kernel_spmd(
                nc, in_maps, core_ids=list(range(NCORES)),
                trace=bool(os.environ.get("CAPS_TRACE")))
            break
        except Exception as e:  # device may need a recovery window
            last_err = e
            import time
            time.sleep(90)
    else:
        raise last_err
    LAST_RESULTS = res
    # core r's vout row (2q + h) holds batch row h*128 + 16r + q
    out = np.empty((B, JD), np.float32)
    for r in range(NCORES):
        vr_ = res.results[r]["vout"].reshape(16, 2, JD)      # (q, h, jd)
        out[16 * r:16 * r + 16] = vr_[:, 0]
        out[128 + 16 * r:128 + 16 * r + 16] = vr_[:, 1]
    return np.ascontiguousarray(out.reshape(B, J, D)[..., None]).astype(
        np.float32)


# revision 11
# speedup vs baseline: 1.1660x; 1.0508x over previous
"""CapsuleLayer dynamic-routing kernel for 8x Trainium2 NeuronCores.

Problem: x (256, 8, 1152) f32, W (1, 1152, 10, 16, 8) f32 ->
v (256, 10, 16, 1) f32 via 4 routing iterations.

u_hat (189 MB) is never materialized; each routing iteration is computed
in factorized form directly from x and W:
    s[b, jd]  = sum_{ck} xT[ck, b] * (c_ij[c, j] * W[c, j, d, k])   (PE)
    M[c, jdk] = sum_b x[b, kc] * v[b, jd]                            (PE)
    agr[c, j] = sum_{dk} W[c, j, d, k] * M[c, j, d, k]               (DVE)

Sharding: the routing state is c-sharded. Core r owns c-block r (128 of
the 1152 input capsules); the 9th block is replicated (every core
computes its full M/agreement so its b_ij stays consistent without a
collective, while its s-contribution is k-striped so the sum is counted
once). b_ij, softmax, Wc, M and the agreement never leave the core; the
only cross-core exchange is ONE AllReduce per iteration of the s partial
sums (256 x 160 fp16, 80 KB), which also serves as the full-batch v
broadcast for the next agreement. W is staged k-major and j-padded to 16
([c, (k, j2, d)]) so it is elementwise-congruent with the M PSUM layout:
the agreement runs directly on PSUM with no layout-fixing copy, and the
per-k Wc slices are contiguous 256-column matmul operands. Matmul
operands are rounded to float32r (TF32) for 4x PE throughput with fp32
PSUM accumulation. End-to-end absmax relative error vs the fp32 oracle
is ~1e-3.
"""
import os
import numpy as np

C, J, D, K = 1152, 10, 16, 8
B, NCORES = 256, 8
BS = B // NCORES
CB = C // 128               # 9 c-blocks; blocks 0..7 sharded, block 8 shared
JD = J * D                  # 160
NIT = 4

_CACHE = {}
LAST_RESULTS = None


def _build(ncores=NCORES, nocc=False):
    """Build + compile the per-core program.

    nocc=True: timing variant for TimelineSim -- collectives are replaced
    by equivalent-size local DMAs that keep the serializing dependency.
    """
    from concourse import bass, bacc, tile, mybir

    f32 = mybir.dt.float32
    f16 = mybir.dt.float16
    f32r = mybir.dt.float32r
    nc = bacc.Bacc("TRN2", target_bir_lowering=False, debug=False,
                   num_devices=ncores)

    # per-core inputs (host-sharded):
    #  xTs:   lhsT for my s-block, [c 128, (k 8, h 2, b 128)]
    #  xT8k:  lhsT for block-8 k=r slice, [c8 128, (h 2, b 128)]
    #  xcs:   M lhsT for my block, [h 2][b 128, (k 8, c 128)]
    #  xcs8:  M lhsT for block 8 (same on all cores)
    #  wpad:  W rows of my block, k-major j-padded (128, (k 8, j2 16, d 16))
    #  w8pad: W rows of block 8, same layout (same on all cores)
    #  w8kr:  my k-slice of block-8 W, (128, (j 10, d 16))
    xTs_d = nc.dram_tensor("xTs", [128, 2048], f32r, kind="ExternalInput").ap()
    xT8k_d = nc.dram_tensor("xT8k", [128, 256], f32r, kind="ExternalInput").ap()
    xcs_d = nc.dram_tensor("xcs", [2, 128, 1024], f32r,
                           kind="ExternalInput").ap()
    xcs8_d = nc.dram_tensor("xcs8", [2, 128, 1024], f32r,
                            kind="ExternalInput").ap()
    wpad_d = nc.dram_tensor("wpad", [128, 2048], f32, kind="ExternalInput").ap()
    w8pad_d = nc.dram_tensor("w8pad", [128, 2048], f32,
                             kind="ExternalInput").ap()
    w8kr_d = nc.dram_tensor("w8kr", [128, JD], f32, kind="ExternalInput").ap()
    vout_d = nc.dram_tensor("vout", [BS, JD], f32, kind="ExternalOutput").ap()

    rg = [list(range(ncores))]

    with tile.TileContext(nc) as tc:
        with (
            tc.tile_pool(name="const", bufs=1) as pc,
            tc.tile_pool(name="work", bufs=3) as pw,
            tc.tile_pool(name="small", bufs=3) as psm,
            tc.tile_pool(name="ps_s", bufs=1, space="PSUM") as pps,
            tc.tile_pool(name="ps_m", bufs=1, space="PSUM") as ppm,
            tc.tile_pool(name="dram", bufs=2, space="DRAM") as pd,
        ):
            mult = mybir.AluOpType.mult
            add = mybir.AluOpType.add

            # ---- persistent tiles ----
            wpad = pc.tile([128, 2048], f32, tag="wpad")
            w8pad = pc.tile([128, 2048], f32, tag="w8pad")
            wcb = pc.tile([128, 2048], f32r, tag="wcb")     # my block's Wc
            wc8 = pc.tile([128, 256], f32r, tag="wc8")      # block-8 k-slice Wc
            xTsr = pc.tile([128, 2048], f32r, tag="xTsr")
            xT8kr = pc.tile([128, 256], f32r, tag="xT8kr")
            xcsr = [pc.tile([128, 1024], f32r, tag=f"xcsr{h}", name=f"xcsr{h}")
                    for h in range(2)]
            xcs8r = [pc.tile([128, 1024], f32r, tag=f"xcs8r{h}",
                             name=f"xcs8r{h}") for h in range(2)]
            vr = pc.tile([128, 512], f32r, tag="vr")        # padded v, 2 halves
            # b_ij for my block (cols 0..9) and block 8 (cols 10..19)
            bij2 = pc.tile([128, 2 * J], f32, tag="bij2")

            w8kr = pc.tile([128, JD], f32, tag="w8kr")
            nc.sync.dma_start(wpad[:], wpad_d)
            nc.sync.dma_start(w8kr[:], w8kr_d)
            nc.sync.dma_start(xTsr[:], xTs_d)
            nc.sync.dma_start(xT8kr[:], xT8k_d)
            for h in range(2):
                nc.sync.dma_start(xcsr[h][:], xcs_d[h])
                nc.sync.dma_start(xcs8r[h][:], xcs8_d[h])
            nc.sync.dma_start(w8pad[:], w8pad_d)
            zeros = pc.tile([128, 512], f32, tag="zeros")
            nc.vector.memset(zeros[:], 0.0)
            nc.vector.tensor_copy(wc8[:, 160:256], zeros[:, :96])
            nc.vector.tensor_copy(vr[:, :512], zeros[:, :512])
            nc.vector.memset(bij2[:], 0.0)

            # my k-slice of block-8 W, viewed (j, d)
            w8kv = w8kr[:].rearrange("p (j d) -> p j d", j=J, d=D)

            for it in range(NIT):
                with nc.named_scope(f"iter{it}"):
                    # ---- c_ij -> Wc for my block + block-8 k-slice ----
                    # wcb pad rows (j2 10..15) are written once from the
                    # zero-padded wpad at it==0 and never touched again.
                    if it == 0:
                        nc.vector.tensor_scalar_mul(wcb[:], wpad[:], 0.1)
                        nc.vector.tensor_scalar_mul(
                            wc8[:].rearrange(
                                "p (j d) -> p j d", j=16, d=16)[:, :J, :],
                            w8kv, 0.1)
                    else:
                        exps = psm.tile([128, 2 * J], f32, tag="exps")
                        mx = psm.tile([128, 2], f32, tag="mx")
                        sumex = psm.tile([128, 2], f32, tag="sumex")
                        rcp = psm.tile([128, 2], f32, tag="rcp")
                        b2v = bij2[:].rearrange("p (g j) -> p g j", g=2, j=J)
                        nc.vector.tensor_reduce(
                            mx[:], b2v, axis=mybir.AxisListType.X,
                            op=mybir.AluOpType.max)
                        nc.vector.tensor_tensor(
                            exps[:].rearrange("p (g j) -> p g j", g=2, j=J),
                            b2v,
                            mx[:].unsqueeze(2).broadcast_to([128, 2, J]),
                            op=mybir.AluOpType.subtract)
                        nc.scalar.activation(
                            exps[:], exps[:],
                            mybir.ActivationFunctionType.Exp)
                        nc.vector.tensor_reduce(
                            sumex[:],
                            exps[:].rearrange("p (g j) -> p g j", g=2, j=J),
                            axis=mybir.AxisListType.X, op=add)
                        nc.vector.reciprocal(rcp[:], sumex[:])
                        # per-k scale: the first slice unblocks the first s
                        # matmul while the rest of the scaling still runs
                        for k in range(K):
                            nc.vector.scalar_tensor_tensor(
                                out=wcb[:, k * 256:(k + 1) * 256].rearrange(
                                    "p (j d) -> p j d", j=16, d=16)[:, :J, :],
                                in0=wpad[:, k * 256:(k + 1) * 256].rearrange(
                                    "p (j d) -> p j d", j=16, d=16)[:, :J, :],
                                scalar=rcp[:, 0:1],
                                in1=exps[:, :J].unsqueeze(2).broadcast_to(
                                    [128, J, D]),
                                op0=mult, op1=mult)
                        nc.vector.scalar_tensor_tensor(
                            out=wc8[:].rearrange(
                                "p (j d) -> p j d", j=16, d=16)[:, :J, :],
                            in0=w8kv, scalar=rcp[:, 1:2],
                            in1=exps[:, J:].unsqueeze(2).broadcast_to(
                                [128, J, D]),
                            op0=mult, op1=mult)

                    # ---- s partial: my block (all k) + block-8 k=r ----
                    ps_s = [pps.tile([128, 256], f32, tag=f"ps_s{h}",
                                     name=f"ps_s{h}") for h in range(2)]
                    for h in range(2):
                        for k in range(K):
                            nc.tensor.matmul(
                                ps_s[h][:],
                                xTsr[:, (k * 2 + h) * 128:(k * 2 + h + 1) * 128],
                                wcb[:, k * 256:(k + 1) * 256],
                                start=(k == 0), stop=False)
                        nc.tensor.matmul(
                            ps_s[h][:], xT8kr[:, h * 128:(h + 1) * 128],
                            wc8[:], start=False, stop=True)

                    # ---- AllReduce s over the 8 c-shards (fp16 payload) ----
                    ssb = psm.tile([128, 2 * JD], f16, tag="ssb")
                    for h in range(2):
                        nc.vector.tensor_copy(
                            ssb[:, h * JD:(h + 1) * JD].rearrange(
                                "p (j d) -> p j d", j=J, d=D),
                            ps_s[h][:].rearrange(
                                "p (j d) -> p j d", j=16, d=16)[:, :J, :])
                    sb_dram = pd.tile([128, 2, JD], f16, tag="sb_dram")
                    for h in range(2):
                        nc.sync.dma_start(sb_dram[:, h],
                                          ssb[:, h * JD:(h + 1) * JD])

                    if it == NIT - 1:
                        # final iteration: ReduceScatter; this core gets batch
                        # rows {h*128 + 16r + q} as (2q + h, jd)
                        rs_dram = pd.tile([BS, JD], f16, tag="rs_dram")
                        if not nocc:
                            nc.gpsimd.collective_compute(
                                "ReduceScatter", add, replica_groups=rg,
                                ins=[sb_dram.opt()], outs=[rs_dram.opt()])
                        else:
                            nc.sync.dma_start(
                                rs_dram[:].rearrange(
                                    "(q h) x -> q h x", q=16, h=2),
                                sb_dram[0:16])
                        rsb = psm.tile([BS, JD], f16, tag="rsb")
                        nc.sync.dma_start(rsb[:], rs_dram[:])
                        tf = psm.tile([BS, JD], f32, tag="tf")
                        sqf = psm.tile([BS, JD], f32, tag="sqf")
                        msf = psm.tile([BS, J], f32, tag="msf")
                        smf = psm.tile([BS, J], f32, tag="smf")
                        onf = psm.tile([BS, J], f32, tag="onf")
                        rcf = psm.tile([BS, J], f32, tag="rcf")
                        fcf = psm.tile([BS, J], f32, tag="fcf")
                        vf = psm.tile([BS, JD], f32, tag="vf")
                        nc.vector.tensor_scalar_add(tf[:], rsb[:], 1e-5)
                        nc.vector.tensor_tensor(sqf[:], tf[:], tf[:], op=mult)
                        nc.vector.tensor_reduce(
                            msf[:],
                            sqf[:].rearrange("p (j d) -> p j d", j=J, d=D),
                            axis=mybir.AxisListType.X, op=add)
                        nc.scalar.sqrt(smf[:], msf[:])
                        nc.vector.tensor_scalar_add(onf[:], msf[:], 1.0)
                        nc.vector.reciprocal(rcf[:], onf[:])
                        nc.vector.tensor_tensor(fcf[:], smf[:], rcf[:],
                                                op=mult)
                        nc.vector.tensor_tensor(
                            vf[:].rearrange("p (j d) -> p j d", j=J, d=D),
                            tf[:].rearrange("p (j d) -> p j d", j=J, d=D),
                            fcf[:].unsqueeze(2).broadcast_to([BS, J, D]),
                            op=mult)
                        nc.sync.dma_start(vout_d, vf[:])
                        continue

                    sr_dram = pd.tile([128, 2, JD], f16, tag="sr_dram",
                                      addr_space="Shared")
                    if not nocc:
                        nc.gpsimd.collective_compute(
                            "AllReduce", add, replica_groups=rg,
                            ins=[sb_dram.opt()], outs=[sr_dram.opt()])
                    else:
                        nc.sync.dma_start(sr_dram[:], sb_dram[:])
                    ssum = psm.tile([128, 2 * JD], f16, tag="ssum")
                    for h in range(2):
                        nc.sync.dma_start(ssum[:, h * JD:(h + 1) * JD],
                                          sr_dram[:, h])

                    # ---- squash over the full batch (128 x 2 halves) ----
                    # leading ops split per half so half 0 starts as soon as
                    # its DMA-completion semaphore fires
                    t = psm.tile([128, 2 * JD], f32, tag="t")
                    sq = psm.tile([128, 2 * JD], f32, tag="sq")
                    magsq = psm.tile([128, 2 * J], f32, tag="magsq")
                    sqm = psm.tile([128, 2 * J], f32, tag="sqm")
                    onep = psm.tile([128, 2 * J], f32, tag="onep")
                    rcp2 = psm.tile([128, 2 * J], f32, tag="rcp2")
                    fct = psm.tile([128, 2 * J], f32, tag="fct")
                    v = psm.tile([128, 2 * JD], f32, tag="v")
                    for h in range(2):
                        hs = slice(h * JD, (h + 1) * JD)
                        nc.vector.tensor_scalar_add(t[:, hs], ssum[:, hs],
                                                    1e-5)
                        nc.vector.tensor_tensor(sq[:, hs], t[:, hs],
                                                t[:, hs], op=mult)
                        nc.vector.tensor_reduce(
                            magsq[:, h * J:(h + 1) * J],
                            sq[:, hs].rearrange("p (j d) -> p j d",
                                                j=J, d=D),
                            axis=mybir.AxisListType.X, op=add)
                    nc.scalar.sqrt(sqm[:], magsq[:])
                    nc.vector.tensor_scalar_add(onep[:], magsq[:], 1.0)
                    nc.vector.reciprocal(rcp2[:], onep[:])
                    nc.vector.tensor_tensor(fct[:], sqm[:], rcp2[:], op=mult)
                    nc.vector.tensor_tensor(
                        v[:].rearrange("p (hj d) -> p hj d", hj=2 * J, d=D),
                        t[:].rearrange("p (hj d) -> p hj d", hj=2 * J, d=D),
                        fct[:].unsqueeze(2).broadcast_to([128, 2 * J, D]),
                        op=mult)

                    # round v into the j-padded f32r tile (2 halves)
                    nc.vector.tensor_copy(
                        vr[:].rearrange("p (h j d) -> p h j d",
                                        h=2, j=16, d=16)[:, :, :J, :],
                        v[:].rearrange("p (h j d) -> p h j d",
                                       h=2, j=J, d=D))

                    # ---- M matmuls + PSUM-direct agreement per block ----
                    # ps_m layout (k, j2 16, d) matches wpad/w8pad exactly,
                    # so the agreement is an elementwise product on PSUM, a
                    # d-reduce, and a 3-step k-tree -- no layout-fixing copy.
                    for lhs, wsrc, bcol, mtag in (
                            (xcsr, wpad, 0, "mb"), (xcs8r, w8pad, 1, "m8")):
                        # two physical PSUM tiles (k 0..3 / k 4..7): PSUM
                        # dependencies are tile-granular, so the first
                        # half's agreement product can start while the
                        # second half's matmuls still run
                        ps_m = [ppm.tile([128, 1024], f32, tag=f"ps_m{g}",
                                         name=f"ps_m{g}_{mtag}")
                                for g in range(2)]
                        for k in range(K):
                            for h in range(2):
                                nc.tensor.matmul(
                                    ps_m[k // 4][:, (k % 4) * 256:
                                                 (k % 4 + 1) * 256],
                                    lhs[h][:, k * 128:(k + 1) * 128],
                                    vr[:, h * 256:(h + 1) * 256],
                                    start=(h == 0), stop=(h == 1))
                        # strided product/reduce over the 10 real j rows
                        # only, split into k-halves so the first half's DVE
                        # ops overlap the second half's matmuls (and the WAR
                        # gap before the next block's matmuls shrinks)
                        pt = pw.tile([128, 2 * 640], f32, tag=f"pt{mtag}",
                                     name=f"pt{mtag}")
                        kj = psm.tile([128, 80], f32, tag=f"kj{mtag}",
                                      name=f"kj{mtag}")
                        for g in range(2):
                            gs = slice(g * 1024, (g + 1) * 1024)
                            nc.vector.tensor_tensor(
                                pt[:, g * 640:(g + 1) * 640].rearrange(
                                    "p (k j d) -> p k j d", k=4, j=J, d=D),
                                wsrc[:, gs].rearrange(
                                    "p (k j2 d) -> p k j2 d",
                                    k=4, j2=16, d=16)[:, :, :J, :],
                                ps_m[g][:].rearrange(
                                    "p (k j2 d) -> p k j2 d",
                                    k=4, j2=16, d=16)[:, :, :J, :],
                                op=mult)
                            nc.vector.tensor_reduce(
                                kj[:, g * 40:(g + 1) * 40],
                                pt[:, g * 640:(g + 1) * 640].rearrange(
                                    "p (kj d) -> p kj d", kj=40, d=16),
                                axis=mybir.AxisListType.X, op=add)
                        t40 = psm.tile([128, 40], f32, tag=f"t40{mtag}",
                                       name=f"t40{mtag}")
                        t20 = psm.tile([128, 20], f32, tag=f"t20{mtag}",
                                       name=f"t20{mtag}")
                        a10 = psm.tile([128, J], f32, tag=f"a10{mtag}",
                                       name=f"a10{mtag}")
                        nc.vector.tensor_tensor(t40[:], kj[:, :40],
                                                kj[:, 40:], op=add)
                        nc.vector.tensor_tensor(t20[:], t40[:, :20],
                                                t40[:, 20:], op=add)
                        nc.vector.tensor_tensor(a10[:], t20[:, :J],
                                                t20[:, J:], op=add)
                        bt = bij2[:, bcol * J:(bcol + 1) * J]
                        nc.vector.tensor_tensor(bt, bt, a10[:], op=add)

    nc.compile()
    return nc


def _tf32(a):
    """Round fp32 -> tf32 bit pattern (round-to-nearest-even on 13 bits)."""
    u = np.ascontiguousarray(a, np.float32).view(np.uint32)
    r = u + np.uint32(0xFFF) + ((u >> np.uint32(13)) & np.uint32(1))
    return (r & np.uint32(0xFFFFE000)).view(np.float32)


def _pad_w(wrows):
    """(128, J, D, K) W rows -> k-major j2=16-padded (128, 2048)."""
    wp = np.zeros((128, K, 16, D), np.float32)
    wp[:, :, :J, :] = wrows.transpose(0, 3, 1, 2)
    return np.ascontiguousarray(wp).reshape(128, 2048)


def _prep_inputs(x, W):
    """Host-side shard + relayout (x is pre-rounded to the tf32 grid the
    tensor engine would use anyway)."""
    x = _tf32(np.ascontiguousarray(x, dtype=np.float32))
    W0 = np.ascontiguousarray(W.reshape(C, J, D, K), dtype=np.float32)
    # block-8 tensors (identical on every core)
    x8 = x[:, :, 1024:1152]                                  # (256, 8, 128)
    xcs8 = np.ascontiguousarray(x8.transpose(0, 1, 2)).reshape(2, 128, 1024)
    w8pad = _pad_w(W0[1024:1152])
    in_maps = []
    for r in range(NCORES):
        xb = x[:, :, r * 128:(r + 1) * 128]                  # (256, 8, 128)
        # xTs[c, (k, h, b)] = x[h*128+b, k, cb_r*128+c]
        xTs = np.ascontiguousarray(
            xb.reshape(2, 128, K, 128).transpose(3, 2, 0, 1)).reshape(128, 2048)
        # xT8k[c8, (h, b)] = x[h*128+b, r, 1024+c8]
        xT8k = np.ascontiguousarray(
            x[:, r, 1024:1152].reshape(2, 128, 128).transpose(2, 0, 1)
        ).reshape(128, 256)
        xcs = np.ascontiguousarray(xb).reshape(2, 128, 1024)
        wpad = _pad_w(W0[r * 128:(r + 1) * 128])
        w8kr = np.ascontiguousarray(W0[1024:1152, :, :, r].reshape(128, JD))
        in_maps.append({
            "xTs": xTs, "xT8k": xT8k, "xcs": xcs, "xcs8": xcs8,
            "wpad": wpad, "w8pad": w8pad, "w8kr": w8kr,
        })
    return in_maps


def kernel(x, W):
    global LAST_RESULTS
    from concourse.bass_utils import run_bass_kernel_spmd

    if "nc" not in _CACHE:
        _CACHE["nc"] = _build()
    nc = _CACHE["nc"]
    in_maps = _prep_inputs(np.asarray(x), np.asarray(W))
    last_err = None
    for attempt in range(3):
        try:
            res = run_bass_kernel_spmd(
                nc, in_maps, core_ids=list(range(NCORES)),
                trace=bool(os.environ.get("CAPS_TRACE")))
            break
        except Exception as e:  # device may need a recovery window
            last_err = e
            import time
            time.sleep(90)
    else:
        raise last_err
    LAST_RESULTS = res
    # core r's vout row (2q + h) holds batch row h*128 + 16r + q
    out = np.empty((B, JD), np.float32)
    for r in range(NCORES):
        vr_ = res.results[r]["vout"].reshape(16, 2, JD)      # (q, h, jd)
        out[16 * r:16 * r + 16] = vr_[:, 0]
        out[128 + 16 * r:128 + 16 * r + 16] = vr_[:, 1]
    return np.ascontiguousarray(out.reshape(B, J, D)[..., None]).astype(
        np.float32)
